# revision 1
# baseline (speedup 1.0000x reference)
"""Differentiable rasterizer on 8 Trainium2 NeuronCores (Bass/Tile).

Math: for each pixel, per stroke, min over bezier samples of squared distance
is computed on the TensorEngine as a quadratic form (pixels as weights,
candidate samples as streaming columns, accumulated negated in PSUM), reduced
on DVE, then alpha-compositing is evaluated in closed form
    C = 1 + sum_s alpha_s * T_s * (c_s - 1),  T_s = prod_{j>s} (1 - alpha_j)
with log-space suffix sums via a triangular matmul. Only ln/exp ACT tables
are used (one ACT function set per kernel):
    2d   = exp(0.5 * ln(-4*m))
    sp   = softplus(arg) = ln(1 + exp(arg)),  arg = 2w - 2d
    w_s  = alpha_s * T_s = exp(arg - sp + psumE),  psumE = -sum_{j>s} sp_j

Work is pruned host-side: for each 16x8 pixel tile and stroke, only samples
with d(center, s) <= dmin_center + 2r (r = tile circumradius) can be the
argmin anywhere in the tile (triangle inequality; exact), plus an absolute
cut  d(center, s) <= r + w + 46  beyond which alpha underflows to 0 in fp32.
Strokes are permuted actives-first per tile (U / widths / colors permuted to
match as per-tile weight data). Tiles are sorted by cost and dealt
round-robin to the 8 cores so the SPMD (single-program) shapes stay tight.
"""
import os
import sys
import time

import numpy as np

sys.path.insert(0, "/opt/trn_rl_repo")

import concourse.bass as bass
import concourse.mybir as mybir
from concourse.tile import TileContext
from concourse.bass_utils import run_bass_kernel_spmd

AF = mybir.ActivationFunctionType
ALU = mybir.AluOpType
F32 = mybir.dt.float32

CS = 512
NSAMP = 50
NSTR = 64
TH, TW = 8, 16  # tile height x width (pixels)
NTY, NTX = CS // TH, CS // TW  # 64 x 32 = 2048 tiles
NTILES = NTY * NTX
NCORES = 8
NSLOTS = NTILES // NCORES  # 256
NGROUPS_FULL = NSLOTS // 8  # 32 groups of 8 slots (4 pairs)
R_TILE = float(np.hypot((TW - 1) / 2.0, (TH - 1) / 2.0))
W_MARGIN = 46.0
DUMMY_PN = 1.0e9

MAX_WAITS = 1

if os.environ.get("DR_LDWOPT", "0") == "1":
    import concourse.bass_utils as _bu

    if not getattr(_bu, "_dr_ldw_patched", False):
        _orig_rc = _bu.run_command

        def _rc(cmd, *a, **kw):
            cmd = [
                "--enable-ldw-opt=true" if c == "--enable-ldw-opt=false" else c
                for c in cmd
            ]
            return _orig_rc(cmd, *a, **kw)

        _bu.run_command = _rc
        _bu._dr_ldw_patched = True


def _split_excess_waits(nc):
    """walrus in this build rejects >1 sync-wait per instruction; move the
    excess onto NoOps inserted before the instruction on the same engine."""
    n_split = 0
    for fn in nc.m.functions:
        for bb in fn.blocks:
            insts = list(bb.instructions)
            out = []
            changed = False
            for inst in insts:
                si = inst.sync_info
                waits = list(si.on_wait) if si is not None and si.on_wait else []
                if len(waits) > MAX_WAITS:
                    changed = True
                    extra = waits[: len(waits) - MAX_WAITS]
                    keep = waits[len(extra):]
                    for i in range(0, len(extra), MAX_WAITS):
                        nop = mybir.InstNoOp(
                            name=f"{inst.name}-ws{n_split}-{i}", ins=[], outs=[]
                        )
                        nop.engine = inst.engine
                        nop.sync_info = mybir.SyncInfo(
                            on_wait=extra[i : i + MAX_WAITS], on_update=[]
                        )
                        out.append(nop)
                    si.on_wait = keep
                    n_split += 1
                out.append(inst)
            if changed:
                bb.instructions[:] = out
    return n_split


def _sample_points(strokes):
    """Mirror the reference's fp32 bezier sampling. [N, S, 2] in pixels."""
    t = np.linspace(0.0, 1.0, NSAMP, dtype=np.float32)[:, None]
    p0, p1, p2, p3 = strokes[:, 0], strokes[:, 1], strokes[:, 2], strokes[:, 3]
    pts = (
        (1 - t[None]) ** 3 * p0[:, None]
        + 3 * (1 - t[None]) ** 2 * t[None] * p1[:, None]
        + 3 * (1 - t[None]) * t[None] ** 2 * p2[:, None]
        + t[None] ** 3 * p3[:, None]
    ).astype(np.float32)
    return pts * np.float32(CS)


def _plan_and_pack(strokes, widths, colors, n_groups):
    """Host-side pruning, tile->core assignment, and input packing."""
    pts = _sample_points(strokes)  # [N,S,2]

    # tile centers
    txc = np.arange(NTX, dtype=np.float64) * TW + (TW - 1) / 2.0
    tyc = np.arange(NTY, dtype=np.float64) * TH + (TH - 1) / 2.0
    cx, cy = np.meshgrid(txc, tyc, indexing="xy")
    centers = np.stack([cx.ravel(), cy.ravel()], -1).astype(np.float32)  # [T,2]

    dc = np.sqrt(
        ((centers[:, None, None, :] - pts[None, :, :, :]) ** 2).sum(-1)
    )  # [T,N,S] float32->64 ok
    dmin_c = dc.min(-1)
    keep = (dc <= dmin_c[:, :, None] + 2 * R_TILE) & (
        dc <= R_TILE + widths[None, :, None] + W_MARGIN
    )  # [T,N,S]
    k_tn = keep.sum(-1)  # candidates per (tile, stroke)
    n_act_t = np.maximum((k_tn > 0).sum(-1), 1)  # [T]
    k_t = np.maximum(k_tn.max(-1), 1)  # [T]

    # sort tiles: cluster by shape so per-slot max-padding stays tight
    order = np.lexsort((n_act_t, -k_t * 64 - n_act_t))  # primary: cost desc
    # per-slot (uniform across cores) shapes
    slot_tiles = order.reshape(NSLOTS, NCORES)  # slot i -> 8 tiles
    n_slot = n_act_t[slot_tiles].max(-1)
    k_slot = k_t[slot_tiles].max(-1)
    # pair-uniform shapes (slots 2p, 2p+1 share dt/reduce)
    n_pair = np.maximum(n_slot[0::2], n_slot[1::2])
    k_pair = np.maximum(k_slot[0::2], k_slot[1::2])

    npairs = NSLOTS // 2
    ksegs = []
    for p in range(npairs):
        n = int(n_pair[p])
        kseg = max(1, 256 // n)
        segs = int(np.ceil(k_pair[p] / kseg))
        kseg = int(np.ceil(k_pair[p] / segs))  # rebalance
        ksegs.append((n, kseg, segs))

    # --- pack per-core candidate tensors ---
    # column layout: per pair, per segment: [slotA strokes x kseg | slotB ...]
    # stroke-major within each slot-half.
    widths2 = (2.0 * widths).astype(np.float32)
    colors_m1 = (colors - 1.0).astype(np.float32)
    U0 = -np.triu(np.ones((NSTR, NSTR), np.float32), 1).T  # U0[j,s]=-1 if j>s

    totw = sum(2 * n * kseg * segs for (n, kseg, segs) in ksegs)
    cand = np.zeros((NCORES, 4, totw), np.float32)
    cand[:, 3, :] = -DUMMY_PN  # default: dummy columns
    ucm_offs = []
    _off = 0
    for (n, kseg, segs) in ksegs:
        ucm_offs.append(_off)
        _off += 2 * n * (2 * n + 6)
    totu = _off
    ucm = np.zeros((NCORES, totu), np.float32)
    w2all = np.zeros((NCORES, 128, npairs), np.float32)

    perms = np.empty((NTILES, NSTR), np.int64)
    for T in range(NTILES):
        act = np.nonzero(k_tn[T] > 0)[0]
        inact = np.nonzero(k_tn[T] == 0)[0]
        perms[T] = np.concatenate([act, inact])

    col_off = 0
    pair_meta = []
    group_meta = []  # (group col_off, group width)
    for p in range(npairs):
        if p % 4 == 0:
            group_meta.append([col_off, 0])
        n, kseg, segs = ksegs[p]
        width_pair = 2 * n * kseg  # columns per segment
        for c in range(NCORES):
            for h in range(2):
                T = slot_tiles[2 * p + h, c]
                perm = perms[T]
                ctr = centers[T]
                # packed-row layout: half h occupies rows/cols [h*n, h*n+n)
                uv = ucm[c, ucm_offs[p] : ucm_offs[p] + 2 * n * (2 * n + 6)]
                uv = uv.reshape(2 * n, 2 * n + 6)
                rows = slice(n * h, n * h + n)
                pn_sel = perm[:n]
                Up = U0[np.ix_(pn_sel, pn_sel)]
                uv[rows, n * h : n * h + n] = Up
                uv[rows, 2 * n + 3 * h : 2 * n + 3 * h + 3] = colors_m1[pn_sel]
                w2all[c, n * h : n * h + n, p] = widths2[pn_sel]
                # candidates (strokes with no candidates keep dummy columns)
                for j in range(n):
                    s_idx = perm[j]
                    if k_tn[T, s_idx] == 0:
                        continue
                    cs = np.nonzero(keep[T, s_idx])[0]
                    q = pts[s_idx, cs].astype(np.float32) - ctr[None, :]
                    for ci in range(len(cs)):
                        seg, r = divmod(ci, kseg)
                        col = (
                            col_off
                            + seg * width_pair
                            + h * (n * kseg)
                            + j * kseg
                            + r
                        )
                        qx, qy = float(q[ci, 0]), float(q[ci, 1])
                        cand[c, 0, col] = 2.0 * qx
                        cand[c, 1, col] = 2.0 * qy
                        cand[c, 2, col] = -1.0
                        cand[c, 3, col] = -(qx * qx + qy * qy)
        pair_meta.append((col_off, n, kseg, segs))
        col_off += width_pair * segs
        group_meta[-1][1] = col_off - group_meta[-1][0]
    assert col_off == totw

    # pixel quad (tile-local, same for every tile): partition p = di*TW+dj
    dj = np.tile(np.arange(TW, dtype=np.float32), TH)
    di = np.repeat(np.arange(TH, dtype=np.float32), TW)
    xl = dj - (TW - 1) / 2.0
    yl = di - (TH - 1) / 2.0
    pixq = np.stack([xl, yl, xl * xl + yl * yl, np.ones(128, np.float32)], 0)
    pixq = pixq.astype(np.float32)  # [4,128]

    ident = np.eye(128, dtype=np.float32)

    in_maps = [
        {
            "cand": cand[c],
            "ucm": ucm[c],
            "w2all": w2all[c],
            "pixq": pixq,
            "ident": ident,
        }
        for c in range(NCORES)
    ]
    plan = {
        "pair_meta": pair_meta,
        "group_meta": group_meta,
        "ucm_offs": ucm_offs,
        "totu": totu,
        "totw": totw,
        "npairs": npairs,
        "n_groups": n_groups,
        "slot_tiles": slot_tiles,
    }
    return in_maps, plan


def _build_program(plan, loop_n=None, dynamic_loop=False):
    n_groups = plan["n_groups"]
    npairs = plan["npairs"]
    totw = plan["totw"]

    nc = bass.Bass("TRN2", target_bir_lowering=False, debug=False,
                   num_devices=NCORES)
    cand = nc.dram_tensor("cand", [4, totw], F32, kind="ExternalInput").ap()
    ucm = nc.dram_tensor("ucm", [plan["totu"]], F32,
                         kind="ExternalInput").ap()
    w2all_d = nc.dram_tensor("w2all", [128, npairs], F32,
                             kind="ExternalInput").ap()
    pixq_d = nc.dram_tensor("pixq", [4, 128], F32, kind="ExternalInput").ap()
    ident_d = nc.dram_tensor("ident", [128, 128], F32,
                             kind="ExternalInput").ap()
    out = nc.dram_tensor("out", [6, NGROUPS_FULL, 512], F32,
                         kind="ExternalOutput").ap()
    niter_d = (
        nc.dram_tensor("niter", [1, 1], mybir.dt.int32, kind="ExternalInput").ap()
        if dynamic_loop
        else None
    )

    with TileContext(nc) as tc:
        with (
            tc.tile_pool(name="const", bufs=1) as constp,
            tc.tile_pool(name="sb", bufs=3) as sb,
            tc.tile_pool(name="sbsmall", bufs=10) as sbs,
            tc.tile_pool(name="cdpool", bufs=3) as cdp,
            tc.tile_pool(name="ps", bufs=2, space="PSUM") as ps,
            tc.tile_pool(name="psdt", bufs=3, space="PSUM") as psdt,
            tc.tile_pool(name="psc", bufs=1, space="PSUM") as psc,
        ):
            pixq_t = constp.tile([4, 128], F32, tag="pixq")
            ident_t = constp.tile([128, 128], F32, tag="ident")
            w2_t = constp.tile([128, npairs], F32, tag="w2")
            nc.sync.dma_start(pixq_t[:], pixq_d[:])
            nc.sync.dma_start(ident_t[:], ident_d[:])
            nc.sync.dma_start(w2_t[:], w2all_d[:])

            import contextlib

            if dynamic_loop:
                nit_t = constp.tile([1, 1], mybir.dt.int32, tag="nit")
                nc.sync.dma_start(nit_t[:], niter_d[:])
                _, (nval,) = nc.values_load_multi_w_load_instructions(
                    nit_t[0:1, 0:1], min_val=1, max_val=8192,
                    skip_runtime_bounds_check=True,
                )
                loop_cm = tc.For_i(0, nval, 1)
            else:
                loop_cm = (
                    tc.For_i(0, loop_n, 1) if loop_n else contextlib.nullcontext()
                )
            _skip_dma = os.environ.get("DR_SKIP_DMA", "")
            _alevel = int(os.environ.get("DR_ALEVEL", "2"))
            with loop_cm:
              CHUNK = 8192

              def stage_a(g):
                  """distance matmuls + per-stroke min + transpose -> mT."""
                  # greedy-pack this group's (pair, seg) blocks into <=CHUNK
                  # column DMAs (blocks are <=512 cols, contiguous in DRAM)
                  blocks = []  # (pair_idx, seg, col_start, width)
                  for jj in range(4):
                      p = g * 4 + jj
                      col0, n, kseg, segs = plan["pair_meta"][p]
                      wseg = 2 * n * kseg
                      for seg in range(segs):
                          blocks.append((jj, seg, col0 + seg * wseg, wseg))
                  chunk_of = {}
                  i = 0
                  ci = 0
                  while i < len(blocks):
                      start = blocks[i][2]
                      w = 0
                      j = i
                      while j < len(blocks) and blocks[j][2] + blocks[j][3] - start <= CHUNK:
                          w = blocks[j][2] + blocks[j][3] - start
                          j += 1
                      ct = cdp.tile([4, CHUNK], F32, tag="cdg")
                      dma_eng = (nc.sync, nc.scalar)[ci % 2]
                      dma_eng.dma_start(ct[:, :w], cand[:, start : start + w])
                      for b in range(i, j):
                          chunk_of[(blocks[b][0], blocks[b][1])] = (
                              ct,
                              blocks[b][2] - start,
                          )
                      i = j
                      ci += 1
                  mT = ps.tile([128, 512], F32, tag="mT")
                  ucm_ts = []
                  mbs = []
                  ns = []
                  for jj in range(4):
                      p = g * 4 + jj
                      col0, n, kseg, segs = plan["pair_meta"][p]
                      wseg = 2 * n * kseg
                      ucm_t = sbs.tile([128, 134], F32, tag="ucm")
                      uoff = plan["ucm_offs"][p]
                      uw = 2 * n + 6
                      nc.sync.dma_start(
                          ucm_t[0 : 2 * n, 0:uw],
                          ucm[uoff : uoff + 2 * n * uw].rearrange(
                              "(r c) -> r c", c=uw
                          ),
                      )
                      ucm_ts.append(ucm_t)
                      mb = sbs.tile([128, 128], F32, tag="mb")
                      for seg in range(segs):
                          ct, off = chunk_of[(jj, seg)]
                          cd = ct[:, off : off + wseg]
                          if _alevel < 0:
                              continue
                          dt = psdt.tile([128, wseg], F32, tag="dt")
                          nc.tensor.matmul(dt[:], pixq_t[:], cd)
                          if _alevel < 1:
                              continue
                          dt_v = dt[:].rearrange("p (a n k) -> p a n k", a=2, n=n)
                          mb_v = mb[:, 0 : 2 * n].rearrange(
                              "p (a c) -> p a c", a=2
                          )
                          if seg == 0:
                              nc.vector.tensor_reduce(
                                  mb_v, dt_v, axis=mybir.AxisListType.X, op=ALU.max
                              )
                          else:
                              tmpr = sbs.tile([128, 128], F32, tag="tmpr")
                              tmp_v = tmpr[:, 0 : 2 * n].rearrange(
                                  "p (a c) -> p a c", a=2
                              )
                              nc.vector.tensor_reduce(
                                  tmp_v, dt_v, axis=mybir.AxisListType.X, op=ALU.max
                              )
                              nc.vector.tensor_tensor(
                                  mb[:, 0 : 2 * n],
                                  mb[:, 0 : 2 * n],
                                  tmpr[:, 0 : 2 * n],
                                  ALU.max,
                              )
                      mbs.append(mb)
                      ns.append(n)
                  if _alevel >= 2:
                      for jj in range(4):
                          n = ns[jj]
                          sl = slice(128 * jj, 128 * (jj + 1))
                          nc.tensor.transpose(
                              mT[0 : 2 * n, sl], mbs[jj][:, 0 : 2 * n], ident_t[:]
                          )
                  return mT, ucm_ts, ns

              def stage_b(g, mT, ucm_ts, ns):
                  """pointwise + compositing matmuls -> out."""
                  lnt = sb.tile([128, 512], F32, tag="lnt")
                  s2t = sb.tile([128, 512], F32, tag="s2t")
                  argt = sb.tile([128, 512], F32, tag="argt")
                  ept = sb.tile([128, 512], F32, tag="ept")
                  spt = sb.tile([128, 512], F32, tag="spt")
                  t1 = sb.tile([128, 512], F32, tag="t1")
                  t2 = sb.tile([128, 512], F32, tag="t2")
                  wA = sb.tile([128, 512], F32, tag="wA")
                  outS = sb.tile([6, 512], F32, tag="outS")
                  pE = ps.tile([128, 512], F32, tag="pE")
                  pC = psc.tile([6, 512], F32, tag="pC")

                  nc.scalar.activation(lnt[:], mT[:], AF.Ln, scale=-4.0)
                  nc.scalar.activation(s2t[:], lnt[:], AF.Exp, scale=0.5)
                  for jj in range(4):
                      p = g * 4 + jj
                      nc.vector.tensor_scalar(
                          argt[:, 128 * jj : 128 * (jj + 1)],
                          s2t[:, 128 * jj : 128 * (jj + 1)],
                          w2_t[:, p : p + 1],
                          -1.0,
                          ALU.subtract,
                          ALU.mult,
                      )
                  nc.scalar.activation(ept[:], argt[:], AF.Exp)
                  nc.scalar.activation(spt[:], ept[:], AF.Ln, bias=1.0)
                  for jj in range(4):
                      n = ns[jj]
                      sl = slice(128 * jj, 128 * (jj + 1))
                      nc.tensor.matmul(
                          pE[0 : 2 * n, sl],
                          ucm_ts[jj][0 : 2 * n, 0 : 2 * n],
                          spt[0 : 2 * n, sl],
                      )
                  nc.vector.tensor_tensor(t1[:], argt[:], spt[:], ALU.subtract)
                  nc.vector.tensor_tensor(t2[:], t1[:], pE[:], ALU.add)
                  nc.scalar.activation(wA[:], t2[:], AF.Exp)
                  for jj in range(4):
                      n = ns[jj]
                      sl = slice(128 * jj, 128 * (jj + 1))
                      nc.tensor.matmul(
                          pC[:, sl],
                          ucm_ts[jj][0 : 2 * n, 2 * n : 2 * n + 6],
                          wA[0 : 2 * n, sl],
                      )
                  nc.scalar.activation(outS[:], pC[:], AF.Identity, bias=1.0)
                  nc.sync.dma_start(out[:, g, :], outS[:])

              ablate = os.environ.get("DR_ABLATE", "")
              if ablate == "a":
                  for g in range(n_groups):
                      stage_a(g)
              elif ablate == "b":
                  for g in range(n_groups):
                      mT = ps.tile([128, 512], F32, tag="mT")
                      ucm_ts = []
                      for jj in range(4):
                          ucm_t = sbs.tile([128, 134], F32, tag="ucm")
                          nc.sync.dma_start(ucm_t[:], ucm[g * 4 + jj])
                          ucm_ts.append(ucm_t)
                      stage_b(g, mT, ucm_ts)
              else:
                  pending = None
                  for g in range(n_groups):
                      cur = stage_a(g)
                      if pending is not None:
                          stage_b(g - 1, *pending)
                      pending = cur
                  stage_b(n_groups - 1, *pending)

    _split_excess_waits(nc)
    return nc


def _scatter(plan, core_outs):
    """Assemble per-core [6, 32, 512] outputs into the [1,3,512,512] canvas."""
    canvas = np.ones((3, CS, CS), np.float32)
    slot_tiles = plan["slot_tiles"]
    for i in range(8 * plan["n_groups"]):
        g, r = divmod(i, 8)
        jj, h = divmod(r, 2)
        for c in range(NCORES):
            T = int(slot_tiles[i, c])
            tyi, txi = divmod(T, NTX)
            blk = core_outs[c][3 * h : 3 * h + 3, g, 128 * jj : 128 * (jj + 1)]
            canvas[
                :, tyi * TH : (tyi + 1) * TH, txi * TW : (txi + 1) * TW
            ] = blk.reshape(3, TH, TW)
    return canvas[None]


_CACHE = {}


def _run(inputs, n_groups, time_reps=0):
    strokes = np.asarray(inputs["strokes"], np.float32)
    widths = np.asarray(inputs["stroke_widths"], np.float32)
    colors = np.asarray(inputs["stroke_colors"], np.float32)
    assert int(inputs["canvas_size"]) == CS

    in_maps, plan = _plan_and_pack(strokes, widths, colors, n_groups)
    nc = _build_program(plan)
    res = run_bass_kernel_spmd(nc, in_maps, list(range(NCORES)))
    outs = [res.results[c]["out"] for c in range(NCORES)]
    return _scatter(plan, outs), plan, nc, in_maps


def kernel(**inputs):
    out, _, _, _ = _run(inputs, NGROUPS_FULL)
    return out


def _make_exec(nc, in_maps):
    import jax
    import jax.numpy as jnp
    from jax.sharding import Mesh, PartitionSpec, NamedSharding
    from jax.experimental.shard_map import shard_map
    from concourse import bass2jax

    bass2jax.install_neuronx_cc_hook()
    partition_name = (
        nc.partition_id_tensor.name if nc.partition_id_tensor else None
    )
    in_names, out_names, out_avals = [], [], []
    for alloc in nc.m.functions[0].allocations:
        if not isinstance(alloc, mybir.MemoryLocationSet):
            continue
        name = alloc.memorylocations[0].name
        if alloc.kind == "ExternalInput":
            if name != partition_name:
                in_names.append(name)
        elif alloc.kind == "ExternalOutput":
            out_names.append(name)
            out_avals.append(
                jax.core.ShapedArray(
                    tuple(alloc.tensor_shape), mybir.dt.np(alloc.dtype)
                )
            )
    n_params = len(in_names)
    all_names = in_names + out_names
    if partition_name is not None:
        all_names = all_names + [partition_name]

    def _body(*args):
        operands = list(args)
        if partition_name is not None:
            operands.append(bass2jax.partition_id_tensor())
        outs = bass2jax._bass_exec_p.bind(
            *operands,
            out_avals=tuple(out_avals),
            in_names=tuple(all_names),
            out_names=tuple(out_names),
            lowering_input_output_aliases=(),
            sim_require_finite=True,
            sim_require_nnan=True,
            nc=nc,
        )
        return tuple(outs)

    devices = jax.devices()[:NCORES]
    mesh = Mesh(np.asarray(devices), ("core",))
    n_outs = len(out_names)
    sharded = jax.jit(
        shard_map(
            _body,
            mesh=mesh,
            in_specs=(PartitionSpec("core"),) * (n_params + n_outs),
            out_specs=(PartitionSpec("core"),) * n_outs,
            check_rep=False,
        ),
        donate_argnums=tuple(range(n_params, n_params + n_outs)),
        keep_unused=True,
    )
    concat_in = [
        jnp.asarray(
            np.concatenate([np.asarray(in_maps[c][n]) for c in range(NCORES)], 0)
        )
        for n in in_names
    ]
    zero_shardings = tuple(
        NamedSharding(mesh, PartitionSpec("core")) for _ in out_avals
    )
    zeros_fn = jax.jit(
        lambda: tuple(
            jnp.zeros((a.shape[0] * NCORES,) + a.shape[1:], a.dtype)
            for a in out_avals
        ),
        out_shardings=zero_shardings,
    )

    def run_once():
        return sharded(*concat_in, *zeros_fn())

    return run_once


def timed_run(inputs, n_groups=NGROUPS_FULL, reps=10, loop_r=65):
    """Device time per kernel via a runtime-trip-count For_i: ONE compiled
    program dispatched with niter=1 and niter=loop_r; per-iteration time is
    (t_R - t_1) / (R - 1) with identical dispatch overhead."""
    import jax

    strokes = np.asarray(inputs["strokes"], np.float32)
    widths = np.asarray(inputs["stroke_widths"], np.float32)
    colors = np.asarray(inputs["stroke_colors"], np.float32)
    in_maps, plan = _plan_and_pack(strokes, widths, colors, n_groups)

    nc = _build_program(plan, dynamic_loop=True)

    def _with_niter(n):
        return [
            {**m, "niter": np.array([[n]], np.int32)} for m in in_maps
        ]

    run1 = _make_exec(nc, _with_niter(1))
    runR = _make_exec(nc, _with_niter(loop_r))

    outs = None
    for _ in range(3):
        outs = run1()
    jax.block_until_ready(outs)
    jax.block_until_ready(runR())

    t1s, tRs = [], []
    for _ in range(reps):
        t0 = time.perf_counter()
        jax.block_until_ready(run1())
        t1s.append(time.perf_counter() - t0)
        t0 = time.perf_counter()
        jax.block_until_ready(runR())
        tRs.append(time.perf_counter() - t0)
    t1 = float(np.median(t1s))
    tR = float(np.median(tRs))
    dt_ns = (tR - t1) / (loop_r - 1) * 1e9
    print(f"  dispatch t1={t1*1e3:.2f}ms tR={tR*1e3:.2f}ms")

    out_global = np.asarray(outs[0])  # [8*6, 32, 512]
    core_outs = [out_global[6 * c : 6 * c + 6] for c in range(NCORES)]
    canvas = _scatter(plan, core_outs)
    return canvas, dt_ns, plan


if __name__ == "__main__":
    n_groups = int(os.environ.get("DR_GROUPS", NGROUPS_FULL))
    import reference as ref

    inputs = ref.setup_inputs()
    t0 = time.time()
    out, plan, nc, in_maps = _run(
        {k: np.asarray(v) if not np.isscalar(v) else v for k, v in inputs.items()},
        n_groups,
    )
    print("kernel wall time:", time.time() - t0)
    expected = np.asarray(ref.reference(**inputs))
    # compare only covered tiles
    cov = np.zeros((CS, CS), bool)
    st = plan["slot_tiles"]
    for i in range(8 * n_groups):
        for c in range(NCORES):
            T = int(st[i, c])
            tyi, txi = divmod(T, NTX)
            cov[tyi * TH : (tyi + 1) * TH, txi * TW : (txi + 1) * TW] = True
    err = np.abs(out - expected)[0][:, cov]
    print(f"covered frac: {cov.mean():.3f}  max abs err: {err.max():.3e}")



# revision 2
# speedup vs baseline: 17.4473x; 17.4473x over previous
"""Differentiable rasterizer on 8 Trainium2 NeuronCores (Bass/Tile) — v2.

Design: exact rectangle-distance band pruning (argmin-capable candidate set
per (tile, stroke)) with alpha-cut margin 5.5 px; non-empty tiles only.
Strokes of ~20-40 tiles are packed as rows of a 128-partition "superslot"
(row 0 is a constant R1 row used to inject 2w via the suffix matmul).
Per slot:
  dt[128px, W] = pixq[7,128]^T @ cand[7,W]      (bf16 hi/lo rows: fp32-
                                                 accurate at 1 PE cyc/col)
  mb[128, 128] = per-class max-reduce over candidate rectangles (min d^2)
  mT = transpose(mb); lnt = Ln(-4 mT); s2t = Exp(.5 lnt) = 2d
  ept = Exp(w2 - s2t)  [per-slot bias AP]; spt = Ln(1 + ept) = softplus
  pE = UU^T @ spt  where UU = (U - I) with w2 injected via the R1 row
  t2 = pE - s2t = arg - sp + suffix(-sp);  wA = Exp(t2)  (bf16)
  pC = colors^T @ wA  (bf16); out = pC + 1
Pointwise ops are batched over quads of 4 slots ([128, 512] tiles).
Compositing order/permutation is folded into host-built UU/colors data.
"""
import os
import sys
import time

import numpy as np
import ml_dtypes

sys.path.insert(0, "/opt/trn_rl_repo")

import concourse.bass as bass
import concourse.mybir as mybir
from concourse.tile import TileContext
from concourse.bass_utils import run_bass_kernel_spmd

AF = mybir.ActivationFunctionType
ALU = mybir.AluOpType
F32 = mybir.dt.float32
BF16 = mybir.dt.bfloat16
NPBF = ml_dtypes.bfloat16

CS = 512
NSAMP = 50
NSTR = 64
TH, TW = 8, 16
NTY, NTX = CS // TH, CS // TW
NCORES = 8
MARGIN = 5.5
KLIST = (1, 2, 4, 8, 16, 32, 64)
MAXTILES = 42          # 3*42 = 126 color rows <= 128
ROWS = 128             # stroke rows per slot (incl R1 at row 0)
R1W2 = float(np.log(np.e - 1.0))

MAX_WAITS = 1


def _split_excess_waits(nc):
    """walrus in this build rejects >1 sync-wait per instruction; move the
    excess onto NoOps inserted before the instruction on the same engine."""
    n_split = 0
    for fn in nc.m.functions:
        for bb in fn.blocks:
            insts = list(bb.instructions)
            out = []
            changed = False
            for inst in insts:
                si = inst.sync_info
                waits = list(si.on_wait) if si is not None and si.on_wait else []
                if len(waits) > MAX_WAITS:
                    changed = True
                    extra = waits[: len(waits) - MAX_WAITS]
                    keep = waits[len(extra):]
                    for i in range(0, len(extra), MAX_WAITS):
                        nop = mybir.InstNoOp(
                            name=f"{inst.name}-ws{n_split}-{i}", ins=[], outs=[]
                        )
                        nop.engine = inst.engine
                        nop.sync_info = mybir.SyncInfo(
                            on_wait=extra[i : i + MAX_WAITS], on_update=[]
                        )
                        out.append(nop)
                    si.on_wait = keep
                    n_split += 1
                out.append(inst)
            if changed:
                bb.instructions[:] = out
    return n_split


def _sample_points(strokes):
    t = np.linspace(0.0, 1.0, NSAMP, dtype=np.float32)[:, None]
    p0, p1, p2, p3 = strokes[:, 0], strokes[:, 1], strokes[:, 2], strokes[:, 3]
    pts = (
        (1 - t[None]) ** 3 * p0[:, None]
        + 3 * (1 - t[None]) ** 2 * t[None] * p1[:, None]
        + 3 * (1 - t[None]) * t[None] ** 2 * p2[:, None]
        + t[None] ** 3 * p3[:, None]
    ).astype(np.float32)
    return pts * np.float32(CS)


def _kclass(k):
    for K in KLIST:
        if k <= K:
            return K
    raise ValueError(k)


def _bf16_hilo(x):
    x = np.asarray(x, np.float32)
    hi = x.astype(NPBF).astype(np.float32)
    lo = (x - hi).astype(np.float32)
    return hi.astype(NPBF), lo.astype(NPBF)


def _plan_and_pack(strokes, widths, colors):
    pts = _sample_points(strokes)  # [N,S,2]

    txc = np.arange(NTX, dtype=np.float64) * TW + (TW - 1) / 2.0
    tyc = np.arange(NTY, dtype=np.float64) * TH + (TH - 1) / 2.0
    cx, cy = np.meshgrid(txc, tyc, indexing="xy")
    centers = np.stack([cx.ravel(), cy.ravel()], -1)  # [T,2]
    qxa = np.abs(centers[:, None, None, 0] - pts[None, :, :, 0])
    qya = np.abs(centers[:, None, None, 1] - pts[None, :, :, 1])
    hx, hy = (TW - 1) / 2.0, (TH - 1) / 2.0
    drect = np.hypot(np.maximum(qxa - hx, 0.0), np.maximum(qya - hy, 0.0))
    dmax = np.hypot(qxa + hx, qya + hy)
    dmax_min = dmax.min(-1)
    keep = (drect <= dmax_min[:, :, None]) & (
        drect <= widths[None, :, None] + MARGIN
    )  # [T,N,S]
    k_tn = keep.sum(-1)

    # tile list: (T, [(s, [cand sample idx])], cost)
    tiles = []
    for T in range(NTY * NTX):
        act = np.nonzero(k_tn[T] > 0)[0]
        if len(act) == 0:
            continue
        entries = [(int(s), np.nonzero(keep[T, s])[0]) for s in act]
        cost = sum(_kclass(len(cs)) for _, cs in entries)
        tiles.append((T, entries, cost))
    tiles.sort(key=lambda x: -x[2])

    # global LPT bin-packing: bins = 8 cores x nslot slots, all
    # interchangeable. Balance columns with a row constraint, spread
    # heavy-class tiles, then group class-profile-similar bins into pairs
    # so per-pair caps (max over 16 instances) stay tight.
    total_rows = sum(len(e) for _, e, _ in tiles)

    def tile_profile(entries):
        cnt = {K: 0 for K in KLIST}
        for s, cs in entries:
            cnt[_kclass(len(cs))] += 1
        return cnt

    profs = {T: tile_profile(e) for T, e, _ in tiles}

    nslot = max(2, -(-total_rows // (NCORES * 112)))
    if nslot % 2:
        nslot += 1
    ok = False
    for _try in range(6):
        nbins = NCORES * nslot
        bins = [
            {"tiles": [], "M": 0, "cols": 0, "cnt": {K: 0 for K in KLIST}}
            for _ in range(nbins)
        ]
        fail = False
        for T, entries, cost in tiles:
            n = len(entries)
            pr = profs[T]
            heavy = sum(pr[K] for K in (16, 32, 64))
            cand = [
                b for b in bins
                if b["M"] + n <= 118 and len(b["tiles"]) < MAXTILES
            ]
            if not cand:
                fail = True
                break
            if heavy:
                b = min(
                    cand,
                    key=lambda b: (
                        sum(b["cnt"][K] for K in (16, 32, 64)),
                        b["cols"], b["M"],
                    ),
                )
            else:
                b = min(cand, key=lambda b: (b["cols"], b["M"]))
            b["tiles"].append((T, entries))
            b["M"] += n
            b["cols"] += cost
            for K in KLIST:
                b["cnt"][K] += pr[K]
        if not fail:
            # group similar bins: sort by class profile desc, chunk by 16
            bins.sort(
                key=lambda b: tuple(-b["cnt"][K] for K in reversed(KLIST))
            )
            npair = nslot // 2
            pair_caps = []
            ok = True
            for p in range(npair):
                grp = bins[16 * p : 16 * (p + 1)]
                caps = {K: 0 for K in KLIST}
                for b in grp:
                    for K in KLIST:
                        caps[K] = max(
                            caps[K], b["cnt"][K] + (1 if K == 1 else 0)
                        )  # +1: R1 row in class 1
                tot = sum(caps.values())
                if tot > ROWS:
                    ok = False
                    break
                caps[1] += ROWS - tot  # dummy rows fill to exactly 128
                W = sum(caps[K] * K for K in KLIST)
                if W > 512:
                    ok = False
                    break
                pair_caps.append(caps)
            if ok:
                # lattice assignment: pair p -> bins[16p:16p+16] dealt to
                # (core, slot 2p / 2p+1)
                core_slots = [[None] * nslot for _ in range(NCORES)]
                for p in range(npair):
                    grp = bins[16 * p : 16 * (p + 1)]
                    for g, b in enumerate(grp):
                        core_slots[g % NCORES][2 * p + g // NCORES] = b
                break
        nslot += 2
    assert ok, "packing failed"

    pair_W = [sum(caps[K] * K for K in KLIST) for caps in pair_caps]
    Wtot = int(sum(2 * w for w in pair_W))

    # ---- build per-core arrays (fp32 staging; cast to bf16 at the end) ----
    widths2 = (2.0 * widths).astype(np.float32)
    cm1 = (colors - 1.0).astype(np.float32)

    candpix = np.zeros((NCORES, 8, 128 + Wtot), np.float32)
    uu = np.zeros((NCORES, 128, nslot * 128), np.float32)
    col_t = np.zeros((NCORES, 128, nslot * 126), np.float32)
    w2 = np.zeros((NCORES, 128, nslot), np.float32)

    # pixel weight rows: [x, x, y, y, x2y2, 1, 1, 0]
    dj = np.tile(np.arange(TW, dtype=np.float32), TH)
    di = np.repeat(np.arange(TH, dtype=np.float32), TW)
    xl = dj - (TW - 1) / 2.0
    yl = di - (TH - 1) / 2.0
    x2y2 = xl * xl + yl * yl
    pixq = np.stack(
        [xl, xl, yl, yl, x2y2, np.ones(128, np.float32), np.ones(128, np.float32),
         np.zeros(128, np.float32)], 0
    )
    for c in range(NCORES):
        candpix[c, :, :128] = pixq

    # class column offsets within a slot (uniform per pair)
    pair_offs = []
    for caps in pair_caps:
        offs = {}
        o = 0
        for K in KLIST:
            offs[K] = o
            o += caps[K] * K
        pair_offs.append(offs)

    # slot -> (tilepos list) for scatter
    slot_tiles_meta = [[None] * nslot for _ in range(NCORES)]

    cand_base = 128
    pair_col0 = []
    o = cand_base
    for p in range(npair):
        pair_col0.append(o)
        o += 2 * pair_W[p]

    for c in range(NCORES):
        for i in range(nslot):
            p, h = divmod(i, 2)
            caps = pair_caps[p]
            offs = pair_offs[p]
            sl = core_slots[c][i]
            col0 = pair_col0[p] + h * pair_W[p]

            # rows: class-major. row index assignment:
            row_base = {}
            rb = 0
            for K in KLIST:
                row_base[K] = rb
                rb += caps[K]
            # R1 = first class-2 row
            next_row = {K: row_base[K] for K in KLIST}

            def place(K):
                r = next_row[K]
                next_row[K] += 1
                assert r < ROWS
                return r

            # R1 row
            r1 = place(1)
            assert r1 == 0
            w2[c, r1, i] = R1W2
            cc = col0 + offs[1] + 0
            candpix[c, 5, cc] = np.float32(-1e-30)

            rows_of = {}  # (tilepos, s) -> row
            tile_ids = []
            for tp, (T, entries) in enumerate(sl["tiles"]):
                tile_ids.append(T)
                for s, cs in entries:
                    K = _kclass(len(cs))
                    r = place(K)
                    rows_of[(tp, s)] = r
                    w2[c, r, i] = widths2[s]
                    # candidate columns
                    q = pts[s, cs].astype(np.float32) - centers[T].astype(
                        np.float32
                    )
                    c2x = 2.0 * q[:, 0]
                    c2y = 2.0 * q[:, 1]
                    cq2 = -(q[:, 0] ** 2 + q[:, 1] ** 2)
                    # pad with duplicates of first candidate
                    npad = K - len(cs)
                    if npad:
                        c2x = np.concatenate([c2x, np.repeat(c2x[:1], npad)])
                        c2y = np.concatenate([c2y, np.repeat(c2y[:1], npad)])
                        cq2 = np.concatenate([cq2, np.repeat(cq2[:1], npad)])
                    xh, xlo = _bf16_hilo(c2x)
                    yh, ylo = _bf16_hilo(c2y)
                    qh, qlo = _bf16_hilo(cq2)
                    cc = col0 + offs[K] + (r - row_base[K]) * K
                    candpix[c, 0, cc : cc + K] = xh
                    candpix[c, 1, cc : cc + K] = xlo
                    candpix[c, 2, cc : cc + K] = yh
                    candpix[c, 3, cc : cc + K] = ylo
                    candpix[c, 4, cc : cc + K] = np.float32(-1.0)
                    candpix[c, 5, cc : cc + K] = qh
                    candpix[c, 6, cc : cc + K] = qlo
                    # colors
                    col_t[c, r, i * 126 + 3 * tp : i * 126 + 3 * tp + 3] = cm1[
                        s
                    ].astype(NPBF)
            # dummy rows: remaining capacity in each class; their columns:
            # q=(0,0) -> m = -x2y2 (row 4 = -1), harmless
            for K in KLIST:
                for r in range(next_row[K], row_base[K] + pair_caps[p][K]):
                    cc = col0 + offs[K] + (r - row_base[K]) * K
                    candpix[c, 4, cc : cc + K] = np.float32(-1.0)

            # UU: [j, s] column s gets -1 for j==s and j after s (same tile)
            U = np.zeros((128, 128), np.float32)
            for tp, (T, entries) in enumerate(sl["tiles"]):
                rr = [
                    (rows_of[(tp, s)], s) for s, _ in entries
                ]  # entries in orig stroke order (act sorted asc)
                for a in range(len(rr)):
                    ra, sa = rr[a]
                    U[ra, ra] = -1.0
                    for b in range(a):
                        rb_, sb_ = rr[b]
                        # sa > sb_: stroke a composites after b -> row ra
                        # contributes -sp to column rb_
                        U[ra, rb_] = -1.0
            U[0, :] = w2[c, :, i]  # R1 row injects w2 (R1 col 0 stays w2[0]=R1W2; harmless)
            U[0, 0] = 0.0
            uu[c, :, i * 128 : (i + 1) * 128] = U
            slot_tiles_meta[c][i] = tile_ids

    ident = np.eye(128, dtype=np.float32)
    candpix_bf = candpix.astype(NPBF)
    col_bf = col_t.astype(NPBF)
    in_maps = [
        {
            "candpix": candpix_bf[c],
            "uu": uu[c],
            "colors": col_bf[c],
            "w2": w2[c],
            "ident": ident,
        }
        for c in range(NCORES)
    ]
    plan = {
        "nslot": nslot,
        "npair": npair,
        "pair_caps": pair_caps,
        "pair_offs": pair_offs,
        "pair_W": pair_W,
        "pair_col0": pair_col0,
        "Wtot": Wtot,
        "slot_tiles": slot_tiles_meta,
    }
    return in_maps, plan


def _build_program(plan, dynamic_loop=False):
    nslot = plan["nslot"]
    npair = plan["npair"]

    nc = bass.Bass("TRN2", target_bir_lowering=False, debug=False,
                   num_devices=NCORES)
    candpix_d = nc.dram_tensor("candpix", [8, 128 + plan["Wtot"]], BF16,
                               kind="ExternalInput").ap()
    uu_d = nc.dram_tensor("uu", [128, nslot * 128], F32,
                          kind="ExternalInput").ap()
    colors_d = nc.dram_tensor("colors", [128, nslot * 126], BF16,
                              kind="ExternalInput").ap()
    w2_d = nc.dram_tensor("w2", [128, nslot], F32, kind="ExternalInput").ap()
    ident_d = nc.dram_tensor("ident", [128, 128], F32,
                             kind="ExternalInput").ap()
    out = nc.dram_tensor("out", [128, nslot * 128], F32,
                         kind="ExternalOutput").ap()
    niter_d = (
        nc.dram_tensor("niter", [1, 1], mybir.dt.int32,
                       kind="ExternalInput").ap()
        if dynamic_loop
        else None
    )

    # quads of slots
    quads = []
    i = 0
    while i < nslot:
        quads.append(list(range(i, min(i + 4, nslot))))
        i += 4

    with TileContext(nc) as tc:
        with (
            tc.tile_pool(name="inp", bufs=2) as inp,
            tc.tile_pool(name="wk", bufs=2) as wk,
            tc.tile_pool(name="psdt", bufs=2, space="PSUM") as psdt,
            tc.tile_pool(name="psmt", bufs=2, space="PSUM") as psmt,
            tc.tile_pool(name="pse", bufs=1, space="PSUM") as pse,
            tc.tile_pool(name="psc", bufs=1, space="PSUM") as psc,
        ):
            import contextlib

            if dynamic_loop:
                nit_t = inp.tile([1, 1], mybir.dt.int32, tag="nit")
                nc.sync.dma_start(nit_t[:], niter_d[:])
                _, (nval,) = nc.values_load_multi_w_load_instructions(
                    nit_t[0:1, 0:1], min_val=1, max_val=8192,
                    skip_runtime_bounds_check=True,
                )
                loop_cm = tc.For_i(0, nval, 1)
            else:
                loop_cm = contextlib.nullcontext()

            with loop_cm:
                cp_t = inp.tile([8, 128 + plan["Wtot"]], BF16, tag="candpix")
                uu_t = inp.tile([128, nslot * 128], F32, tag="uu")
                col_tt = inp.tile([128, nslot * 126], BF16, tag="colors")
                w2_t = inp.tile([128, nslot], F32, tag="w2")
                ident_t = inp.tile([128, 128], F32, tag="ident")
                nc.sync.dma_start(cp_t[:], candpix_d[:])
                nc.sync.dma_start(w2_t[:], w2_d[:])
                nc.sync.dma_start(ident_t[:], ident_d[:])
                # split uu/colors halves on the two hwdge queues
                half = (nslot // 2) * 128
                nc.sync.dma_start(uu_t[:, :half], uu_d[:, :half])
                nc.scalar.dma_start(uu_t[:, half:], uu_d[:, half:])
                halfc = (nslot // 2) * 126
                nc.scalar.dma_start(col_tt[:, :halfc], colors_d[:, :halfc])
                nc.scalar.dma_start(col_tt[:, halfc:], colors_d[:, halfc:])

                def emit_pair(p):
                    """distance matmuls + class reduces + transposes for
                    pair p. Returns mb tile."""
                    W = plan["pair_W"][p]
                    caps = plan["pair_caps"][p]
                    offs = plan["pair_offs"][p]
                    col0 = plan["pair_col0"][p]
                    dt = psdt.tile([128, 1024], F32, tag="dt")
                    for h in range(2):
                        nc.tensor.matmul(
                            dt[:, 512 * h : 512 * h + W],
                            cp_t[0:7, 0:128],
                            cp_t[0:7, col0 + h * W : col0 + (h + 1) * W],
                        )
                    mb = wk.tile([128, 256], F32, tag="mb")
                    mb_v = mb[:].rearrange("p (a c) -> p a c", a=2)
                    dt_v = dt[:].rearrange("p (a b) -> p a b", a=2)
                    rb = 0
                    for K in KLIST:
                        cap = caps[K]
                        if cap == 0:
                            continue
                        src = dt_v[:, :, offs[K] : offs[K] + cap * K].rearrange(
                            "p a (n k) -> p a n k", k=K
                        )
                        nc.vector.tensor_reduce(
                            mb_v[:, :, rb : rb + cap], src,
                            axis=mybir.AxisListType.X, op=ALU.max,
                        )
                        rb += cap
                    return mb

                def emit_quad(q, qi):
                    qn = len(q)
                    mT = psmt.tile([128, 512], F32, tag="mT")
                    mbs = []
                    for j in range(0, qn, 2):
                        p = q[j] // 2
                        mbs.append(emit_pair(p))
                    for j in range(qn):
                        mb = mbs[j // 2]
                        nc.tensor.transpose(
                            mT[:, 128 * j : 128 * (j + 1)],
                            mb[:, 128 * (j % 2) : 128 * (j % 2 + 1)],
                            ident_t[:],
                        )
                    wq = 128 * qn
                    lnt = wk.tile([128, 512], F32, tag="lnt")
                    s2t = wk.tile([128, 512], F32, tag="s2t")
                    ept = wk.tile([128, 512], F32, tag="ept")
                    spt = wk.tile([128, 512], F32, tag="spt")
                    t2 = wk.tile([128, 512], F32, tag="t2")
                    wA = wk.tile([128, 512], BF16, tag="wA")
                    outS = wk.tile([128, 512], F32, tag="outS")
                    pE = pse.tile([128, 512], F32, tag="pE")
                    pC = psc.tile([128, 512], F32, tag="pC")

                    nc.scalar.activation(lnt[:, :wq], mT[:, :wq], AF.Ln,
                                         scale=-4.0)
                    nc.scalar.activation(s2t[:, :wq], lnt[:, :wq], AF.Exp,
                                         scale=0.5)
                    for j, i in enumerate(q):
                        nc.scalar.activation(
                            ept[:, 128 * j : 128 * (j + 1)],
                            s2t[:, 128 * j : 128 * (j + 1)],
                            AF.Exp, scale=-1.0, bias=w2_t[:, i : i + 1],
                        )
                    nc.scalar.activation(spt[:, :wq], ept[:, :wq], AF.Ln,
                                         bias=1.0)
                    for j, i in enumerate(q):
                        nc.tensor.matmul(
                            pE[:, 128 * j : 128 * (j + 1)],
                            uu_t[:, 128 * i : 128 * (i + 1)],
                            spt[:, 128 * j : 128 * (j + 1)],
                        )
                    nc.vector.tensor_tensor(t2[:, :wq], pE[:, :wq],
                                            s2t[:, :wq], ALU.subtract)
                    nc.scalar.activation(wA[:, :wq], t2[:, :wq], AF.Exp)
                    for j, i in enumerate(q):
                        nc.tensor.matmul(
                            pC[0:126, 128 * j : 128 * (j + 1)],
                            col_tt[:, 126 * i : 126 * (i + 1)],
                            wA[:, 128 * j : 128 * (j + 1)],
                        )
                    nc.scalar.activation(outS[0:126, :wq], pC[0:126, :wq],
                                         AF.Identity, bias=1.0)
                    nc.sync.dma_start(
                        out[0:126, 128 * q[0] : 128 * q[0] + wq],
                        outS[0:126, :wq],
                    )

                for qi, q in enumerate(quads):
                    emit_quad(q, qi)

    _split_excess_waits(nc)
    return nc


def _scatter(plan, core_outs):
    canvas = np.ones((3, CS, CS), np.float32)
    for c in range(NCORES):
        for i in range(plan["nslot"]):
            tiles = plan["slot_tiles"][c][i]
            if not tiles:
                continue
            blk = core_outs[c][:, 128 * i : 128 * (i + 1)]
            for tp, T in enumerate(tiles):
                tyi, txi = divmod(T, NTX)
                canvas[
                    :, tyi * TH : (tyi + 1) * TH, txi * TW : (txi + 1) * TW
                ] = blk[3 * tp : 3 * tp + 3].reshape(3, TH, TW)
    return canvas[None]


def _run(inputs):
    strokes = np.asarray(inputs["strokes"], np.float32)
    widths = np.asarray(inputs["stroke_widths"], np.float32)
    colors = np.asarray(inputs["stroke_colors"], np.float32)
    assert int(inputs["canvas_size"]) == CS

    in_maps, plan = _plan_and_pack(strokes, widths, colors)
    nc = _build_program(plan)
    res = run_bass_kernel_spmd(nc, in_maps, list(range(NCORES)))
    outs = [res.results[c]["out"] for c in range(NCORES)]
    return _scatter(plan, outs), plan, nc, in_maps


def kernel(**inputs):
    out, _, _, _ = _run(inputs)
    return out


def _make_exec(nc, in_maps):
    import jax
    import jax.numpy as jnp
    from jax.sharding import Mesh, PartitionSpec, NamedSharding
    from jax.experimental.shard_map import shard_map
    from concourse import bass2jax

    bass2jax.install_neuronx_cc_hook()
    partition_name = (
        nc.partition_id_tensor.name if nc.partition_id_tensor else None
    )
    in_names, out_names, out_avals = [], [], []
    for alloc in nc.m.functions[0].allocations:
        if not isinstance(alloc, mybir.MemoryLocationSet):
            continue
        name = alloc.memorylocations[0].name
        if alloc.kind == "ExternalInput":
            if name != partition_name:
                in_names.append(name)
        elif alloc.kind == "ExternalOutput":
            out_names.append(name)
            out_avals.append(
                jax.core.ShapedArray(
                    tuple(alloc.tensor_shape), mybir.dt.np(alloc.dtype)
                )
            )
    n_params = len(in_names)
    all_names = in_names + out_names
    if partition_name is not None:
        all_names = all_names + [partition_name]

    def _body(*args):
        operands = list(args)
        if partition_name is not None:
            operands.append(bass2jax.partition_id_tensor())
        outs = bass2jax._bass_exec_p.bind(
            *operands,
            out_avals=tuple(out_avals),
            in_names=tuple(all_names),
            out_names=tuple(out_names),
            lowering_input_output_aliases=(),
            sim_require_finite=True,
            sim_require_nnan=True,
            nc=nc,
        )
        return tuple(outs)

    devices = jax.devices()[:NCORES]
    mesh = Mesh(np.asarray(devices), ("core",))
    n_outs = len(out_names)
    sharded = jax.jit(
        shard_map(
            _body,
            mesh=mesh,
            in_specs=(PartitionSpec("core"),) * (n_params + n_outs),
            out_specs=(PartitionSpec("core"),) * n_outs,
            check_rep=False,
        ),
        donate_argnums=tuple(range(n_params, n_params + n_outs)),
        keep_unused=True,
    )
    concat_in = [
        jnp.asarray(
            np.concatenate([np.asarray(in_maps[c][n]) for c in range(NCORES)], 0)
        )
        for n in in_names
    ]
    zero_shardings = tuple(
        NamedSharding(mesh, PartitionSpec("core")) for _ in out_avals
    )
    zeros_fn = jax.jit(
        lambda: tuple(
            jnp.zeros((a.shape[0] * NCORES,) + a.shape[1:], a.dtype)
            for a in out_avals
        ),
        out_shardings=zero_shardings,
    )

    def run_once():
        return sharded(*concat_in, *zeros_fn())

    return run_once


def timed_run(inputs, reps=10, loop_r=65):
    import jax

    strokes = np.asarray(inputs["strokes"], np.float32)
    widths = np.asarray(inputs["stroke_widths"], np.float32)
    colors = np.asarray(inputs["stroke_colors"], np.float32)
    in_maps, plan = _plan_and_pack(strokes, widths, colors)

    nc = _build_program(plan, dynamic_loop=True)

    def _with_niter(n):
        return [
            {**m, "niter": np.array([[n]], np.int32)} for m in in_maps
        ]

    run1 = _make_exec(nc, _with_niter(1))
    runR = _make_exec(nc, _with_niter(loop_r))

    outs = None
    for _ in range(3):
        outs = run1()
    jax.block_until_ready(outs)
    jax.block_until_ready(runR())

    t1s, tRs = [], []
    for _ in range(reps):
        t0 = time.perf_counter()
        jax.block_until_ready(run1())
        t1s.append(time.perf_counter() - t0)
        t0 = time.perf_counter()
        jax.block_until_ready(runR())
        tRs.append(time.perf_counter() - t0)
    t1 = float(np.median(t1s))
    tR = float(np.median(tRs))
    dt_ns = (tR - t1) / (loop_r - 1) * 1e9
    print(f"  dispatch t1={t1*1e3:.2f}ms tR={tR*1e3:.2f}ms")

    out_global = np.asarray(outs[0])  # [8*128, nslot*128]
    core_outs = [out_global[128 * c : 128 * (c + 1)] for c in range(NCORES)]
    canvas = _scatter(plan, core_outs)
    return canvas, dt_ns, plan


if __name__ == "__main__":
    import reference as ref

    inputs = ref.setup_inputs()
    np_inputs = {
        "strokes": np.asarray(inputs["strokes"]),
        "stroke_widths": np.asarray(inputs["stroke_widths"]),
        "stroke_colors": np.asarray(inputs["stroke_colors"]),
        "canvas_size": inputs["canvas_size"],
    }
    t0 = time.time()
    out, plan, nc, in_maps = _run(np_inputs)
    print("kernel wall time:", time.time() - t0)
    expected = np.asarray(ref.reference(**inputs))
    err = np.abs(out - expected)
    scale = np.abs(expected).max()
    print(f"nslot={plan['nslot']} pair_W={plan['pair_W']}")
    print(f"max abs err: {err.max():.3e}")
    print(f"Relative error: {err.max()/scale:.6e}")


# revision 4
# speedup vs baseline: 25.7905x; 1.4782x over previous
"""Differentiable rasterizer on 8 Trainium2 NeuronCores (Bass/Tile) — v2.

Design: exact rectangle-distance band pruning (argmin-capable candidate set
per (tile, stroke)) with alpha-cut margin 5.5 px; non-empty tiles only.
Strokes of ~20-40 tiles are packed as rows of a 128-partition "superslot"
(row 0 is a constant R1 row used to inject 2w via the suffix matmul).
Per slot:
  dt[128px, W] = pixq[7,128]^T @ cand[7,W]      (bf16 hi/lo rows: fp32-
                                                 accurate at 1 PE cyc/col)
  mb[128, 128] = per-class max-reduce over candidate rectangles (min d^2)
  mT = transpose(mb); lnt = Ln(-4 mT); s2t = Exp(.5 lnt) = 2d
  ept = Exp(w2 - s2t)  [per-slot bias AP]; spt = Ln(1 + ept) = softplus
  pE = UU^T @ spt  where UU = (U - I) with w2 injected via the R1 row
  t2 = pE - s2t = arg - sp + suffix(-sp);  wA = Exp(t2)  (bf16)
  pC = colors^T @ wA  (bf16); out = pC + 1
Pointwise ops are batched over quads of 4 slots ([128, 512] tiles).
Compositing order/permutation is folded into host-built UU/colors data.
"""
import os
import sys
import time

import numpy as np
import ml_dtypes

sys.path.insert(0, "/opt/trn_rl_repo")

import concourse.bass as bass
import concourse.mybir as mybir
from concourse.tile import TileContext
from concourse.bass_utils import run_bass_kernel_spmd

AF = mybir.ActivationFunctionType
ALU = mybir.AluOpType
F32 = mybir.dt.float32
BF16 = mybir.dt.bfloat16
NPBF = ml_dtypes.bfloat16

CS = 512
NSAMP = 50
NSTR = 64
TH, TW = 8, 16
NTY, NTX = CS // TH, CS // TW
NCORES = 8
MARGIN = float(os.environ.get("DR_MARGIN", 4.0))
KLIST = (1, 2, 4, 8, 16, 24, 32, 48, 64)
MAXTILES = 42          # 3*42 = 126 color rows <= 128
ROWS = 128             # stroke rows per slot (incl R1 at row 0)
R1W2 = float(np.log(np.e - 1.0))

MAX_WAITS = 1


def _split_excess_waits(nc):
    """walrus in this build rejects >1 sync-wait per instruction; move the
    excess onto NoOps inserted before the instruction on the same engine."""
    n_split = 0
    for fn in nc.m.functions:
        for bb in fn.blocks:
            insts = list(bb.instructions)
            out = []
            changed = False
            for inst in insts:
                si = inst.sync_info
                waits = list(si.on_wait) if si is not None and si.on_wait else []
                if len(waits) > MAX_WAITS:
                    changed = True
                    extra = waits[: len(waits) - MAX_WAITS]
                    keep = waits[len(extra):]
                    for i in range(0, len(extra), MAX_WAITS):
                        nop = mybir.InstNoOp(
                            name=f"{inst.name}-ws{n_split}-{i}", ins=[], outs=[]
                        )
                        nop.engine = inst.engine
                        nop.sync_info = mybir.SyncInfo(
                            on_wait=extra[i : i + MAX_WAITS], on_update=[]
                        )
                        out.append(nop)
                    si.on_wait = keep
                    n_split += 1
                out.append(inst)
            if changed:
                bb.instructions[:] = out
    return n_split


def _sample_points(strokes):
    t = np.linspace(0.0, 1.0, NSAMP, dtype=np.float32)[:, None]
    p0, p1, p2, p3 = strokes[:, 0], strokes[:, 1], strokes[:, 2], strokes[:, 3]
    pts = (
        (1 - t[None]) ** 3 * p0[:, None]
        + 3 * (1 - t[None]) ** 2 * t[None] * p1[:, None]
        + 3 * (1 - t[None]) * t[None] ** 2 * p2[:, None]
        + t[None] ** 3 * p3[:, None]
    ).astype(np.float32)
    return pts * np.float32(CS)


def _kclass(k):
    for K in KLIST:
        if k <= K:
            return K
    raise ValueError(k)


def _bf16_hilo(x):
    x = np.asarray(x, np.float32)
    hi = x.astype(NPBF).astype(np.float32)
    lo = (x - hi).astype(np.float32)
    return hi.astype(NPBF), lo.astype(NPBF)


def _plan_and_pack(strokes, widths, colors):
    pts = _sample_points(strokes)  # [N,S,2]

    txc = np.arange(NTX, dtype=np.float64) * TW + (TW - 1) / 2.0
    tyc = np.arange(NTY, dtype=np.float64) * TH + (TH - 1) / 2.0
    cx, cy = np.meshgrid(txc, tyc, indexing="xy")
    centers = np.stack([cx.ravel(), cy.ravel()], -1)  # [T,2]
    qxa = np.abs(centers[:, None, None, 0] - pts[None, :, :, 0])
    qya = np.abs(centers[:, None, None, 1] - pts[None, :, :, 1])
    hx, hy = (TW - 1) / 2.0, (TH - 1) / 2.0
    drect = np.hypot(np.maximum(qxa - hx, 0.0), np.maximum(qya - hy, 0.0))
    dmax = np.hypot(qxa + hx, qya + hy)
    dmax_min = dmax.min(-1)
    keep = (drect <= dmax_min[:, :, None]) & (
        drect <= widths[None, :, None] + MARGIN
    )  # [T,N,S]
    k_tn = keep.sum(-1)

    # tile list: (T, [(s, [cand sample idx])], cost)
    tiles = []
    for T in range(NTY * NTX):
        act = np.nonzero(k_tn[T] > 0)[0]
        if len(act) == 0:
            continue
        entries = [(int(s), np.nonzero(keep[T, s])[0]) for s in act]
        cost = sum(_kclass(len(cs)) for _, cs in entries)
        tiles.append((T, entries, cost))
    tiles.sort(key=lambda x: -x[2])

    # global LPT bin-packing: bins = 8 cores x nslot slots, all
    # interchangeable. Balance columns with a row constraint, spread
    # heavy-class tiles, then group class-profile-similar bins into pairs
    # so per-pair caps (max over 16 instances) stay tight.
    total_rows = sum(len(e) for _, e, _ in tiles)

    def tile_profile(entries):
        cnt = {K: 0 for K in KLIST}
        for s, cs in entries:
            cnt[_kclass(len(cs))] += 1
        return cnt

    profs = {T: tile_profile(e) for T, e, _ in tiles}

    class_tot = {K: 0 for K in KLIST}
    for T, e, _ in tiles:
        for K in KLIST:
            class_tot[K] += profs[T][K]

    def try_pack(nslot, rowcap, slack_lo, slack_hi):
        nbins = NCORES * nslot
        target = {
            K: -(-class_tot[K] // nbins) + (slack_lo if K <= 2 else slack_hi)
            for K in KLIST
        }
        bins = [
            {"tiles": [], "M": 0, "cols": 0, "cnt": {K: 0 for K in KLIST}}
            for _ in range(nbins)
        ]
        for T, entries, cost in tiles:
            n = len(entries)
            pr = profs[T]
            cand = [
                b for b in bins
                if b["M"] + n <= rowcap and len(b["tiles"]) < MAXTILES
            ]
            if not cand:
                return None

            def viol(b):
                return sum(
                    max(0, b["cnt"][K] + pr[K] - target[K]) * K
                    for K in KLIST
                    if pr[K]
                )

            b = min(cand, key=lambda b: (viol(b), b["cols"], b["M"]))
            b["tiles"].append((T, entries))
            b["M"] += n
            b["cols"] += cost
            for K in KLIST:
                b["cnt"][K] += pr[K]
        # group similar bins: sort by class profile desc, chunk by 16
        bins.sort(
            key=lambda b: tuple(-b["cnt"][K] for K in reversed(KLIST))
        )
        npair = nslot // 2
        pair_caps = []
        for p in range(npair):
            grp = bins[16 * p : 16 * (p + 1)]
            caps = {K: 0 for K in KLIST}
            for b in grp:
                for K in KLIST:
                    caps[K] = max(
                        caps[K], b["cnt"][K] + (1 if K == 1 else 0)
                    )  # +1: R1 row in class 1
            tot = sum(caps.values())
            if tot > ROWS:
                return None
            caps[1] += ROWS - tot  # dummy rows fill to exactly 128
            W = sum(caps[K] * K for K in KLIST)
            if W > 512:
                return None
            pair_caps.append(caps)
        core_slots = [[None] * nslot for _ in range(NCORES)]
        for p in range(npair):
            grp = bins[16 * p : 16 * (p + 1)]
            for g, b in enumerate(grp):
                core_slots[g % NCORES][2 * p + g // NCORES] = b
        return pair_caps, core_slots

    nslot = max(2, -(-total_rows // (NCORES * 124)))
    if nslot % 2:
        nslot += 1
    res = None
    for _try in range(6):
        for rowcap in (124, 122, 119, 116):
            for slack_lo, slack_hi in ((1, 1), (2, 1), (1, 0), (3, 2)):
                res = try_pack(nslot, rowcap, slack_lo, slack_hi)
                if res is not None:
                    break
            if res is not None:
                break
        if res is not None:
            break
        nslot += 2
    assert res is not None, "packing failed"
    pair_caps, core_slots = res
    npair = nslot // 2

    pair_W = [sum(caps[K] * K for K in KLIST) for caps in pair_caps]
    Wtot = int(sum(2 * w for w in pair_W))

    # ---- build per-core arrays (fp32 staging; cast to bf16 at the end) ----
    widths2 = (2.0 * widths).astype(np.float32)
    cm1 = (colors - 1.0).astype(np.float32)

    candpix = np.zeros((NCORES, 8, 128 + Wtot), np.float32)
    uu = np.zeros((NCORES, 128, nslot * 128), np.float32)
    col_t = np.zeros((NCORES, 128, nslot * 126), np.float32)
    w2 = np.zeros((NCORES, 128, nslot), np.float32)

    # pixel weight rows: [x, x, y, y, x2y2, 1, 1, 0]
    dj = np.tile(np.arange(TW, dtype=np.float32), TH)
    di = np.repeat(np.arange(TH, dtype=np.float32), TW)
    xl = dj - (TW - 1) / 2.0
    yl = di - (TH - 1) / 2.0
    x2y2 = xl * xl + yl * yl
    pixq = np.stack(
        [xl, xl, yl, yl, x2y2, np.ones(128, np.float32), np.ones(128, np.float32),
         np.zeros(128, np.float32)], 0
    )
    for c in range(NCORES):
        candpix[c, :, :128] = pixq

    # class column offsets within a slot (uniform per pair)
    pair_offs = []
    for caps in pair_caps:
        offs = {}
        o = 0
        for K in KLIST:
            offs[K] = o
            o += caps[K] * K
        pair_offs.append(offs)

    # slot -> (tilepos list) for scatter
    slot_tiles_meta = [[None] * nslot for _ in range(NCORES)]

    cand_base = 128
    pair_col0 = []
    o = cand_base
    for p in range(npair):
        pair_col0.append(o)
        o += 2 * pair_W[p]

    for c in range(NCORES):
        for i in range(nslot):
            p, h = divmod(i, 2)
            caps = pair_caps[p]
            offs = pair_offs[p]
            sl = core_slots[c][i]
            col0 = pair_col0[p] + h * pair_W[p]

            # rows: class-major. row index assignment:
            row_base = {}
            rb = 0
            for K in KLIST:
                row_base[K] = rb
                rb += caps[K]
            # R1 = first class-2 row
            next_row = {K: row_base[K] for K in KLIST}

            def place(K):
                r = next_row[K]
                next_row[K] += 1
                assert r < ROWS
                return r

            # R1 row
            r1 = place(1)
            assert r1 == 0
            w2[c, r1, i] = R1W2
            cc = col0 + offs[1] + 0
            candpix[c, 5, cc] = np.float32(-1e-30)

            rows_of = {}  # (tilepos, s) -> row
            tile_ids = []
            for tp, (T, entries) in enumerate(sl["tiles"]):
                tile_ids.append(T)
                for s, cs in entries:
                    K = _kclass(len(cs))
                    r = place(K)
                    rows_of[(tp, s)] = r
                    w2[c, r, i] = widths2[s]
                    # candidate columns
                    q = pts[s, cs].astype(np.float32) - centers[T].astype(
                        np.float32
                    )
                    c2x = 2.0 * q[:, 0]
                    c2y = 2.0 * q[:, 1]
                    cq2 = -(q[:, 0] ** 2 + q[:, 1] ** 2)
                    # pad with duplicates of first candidate
                    npad = K - len(cs)
                    if npad:
                        c2x = np.concatenate([c2x, np.repeat(c2x[:1], npad)])
                        c2y = np.concatenate([c2y, np.repeat(c2y[:1], npad)])
                        cq2 = np.concatenate([cq2, np.repeat(cq2[:1], npad)])
                    xh, xlo = _bf16_hilo(c2x)
                    yh, ylo = _bf16_hilo(c2y)
                    qh, qlo = _bf16_hilo(cq2)
                    cc = col0 + offs[K] + (r - row_base[K]) * K
                    candpix[c, 0, cc : cc + K] = xh
                    candpix[c, 1, cc : cc + K] = xlo
                    candpix[c, 2, cc : cc + K] = yh
                    candpix[c, 3, cc : cc + K] = ylo
                    candpix[c, 4, cc : cc + K] = np.float32(-1.0)
                    candpix[c, 5, cc : cc + K] = qh
                    candpix[c, 6, cc : cc + K] = qlo
                    # colors
                    col_t[c, r, i * 126 + 3 * tp : i * 126 + 3 * tp + 3] = cm1[
                        s
                    ].astype(NPBF)
            # dummy rows: remaining capacity in each class; their columns:
            # q=(0,0) -> m = -x2y2 (row 4 = -1), harmless
            for K in KLIST:
                for r in range(next_row[K], row_base[K] + pair_caps[p][K]):
                    cc = col0 + offs[K] + (r - row_base[K]) * K
                    candpix[c, 4, cc : cc + K] = np.float32(-1.0)

            # UU: [j, s] column s gets -1 for j==s and j after s (same tile)
            U = np.zeros((128, 128), np.float32)
            for tp, (T, entries) in enumerate(sl["tiles"]):
                rr = [
                    (rows_of[(tp, s)], s) for s, _ in entries
                ]  # entries in orig stroke order (act sorted asc)
                for a in range(len(rr)):
                    ra, sa = rr[a]
                    U[ra, ra] = -1.0
                    for b in range(a):
                        rb_, sb_ = rr[b]
                        # sa > sb_: stroke a composites after b -> row ra
                        # contributes -sp to column rb_
                        U[ra, rb_] = -1.0
            U[0, :] = w2[c, :, i]  # R1 row injects w2 (R1 col 0 stays w2[0]=R1W2; harmless)
            U[0, 0] = 0.0
            uu[c, :, i * 128 : (i + 1) * 128] = U
            slot_tiles_meta[c][i] = tile_ids

    ident = np.eye(128, dtype=np.float32)
    candpix_bf = candpix.astype(NPBF)
    col_bf = col_t.astype(NPBF)
    in_maps = [
        {
            "candpix": candpix_bf[c],
            "uu": uu[c],
            "colors": col_bf[c],
            "w2": w2[c],
            "ident": ident,
        }
        for c in range(NCORES)
    ]
    plan = {
        "nslot": nslot,
        "npair": npair,
        "pair_caps": pair_caps,
        "pair_offs": pair_offs,
        "pair_W": pair_W,
        "pair_col0": pair_col0,
        "Wtot": Wtot,
        "slot_tiles": slot_tiles_meta,
    }
    return in_maps, plan


def _build_program(plan, dynamic_loop=False):
    nslot = plan["nslot"]
    npair = plan["npair"]

    nc = bass.Bass("TRN2", target_bir_lowering=False, debug=False,
                   num_devices=NCORES)
    candpix_d = nc.dram_tensor("candpix", [8, 128 + plan["Wtot"]], BF16,
                               kind="ExternalInput").ap()
    uu_d = nc.dram_tensor("uu", [128, nslot * 128], F32,
                          kind="ExternalInput").ap()
    colors_d = nc.dram_tensor("colors", [128, nslot * 126], BF16,
                              kind="ExternalInput").ap()
    w2_d = nc.dram_tensor("w2", [128, nslot], F32, kind="ExternalInput").ap()
    ident_d = nc.dram_tensor("ident", [128, 128], F32,
                             kind="ExternalInput").ap()
    out = nc.dram_tensor("out", [128, nslot * 128], F32,
                         kind="ExternalOutput").ap()
    niter_d = (
        nc.dram_tensor("niter", [1, 1], mybir.dt.int32,
                       kind="ExternalInput").ap()
        if dynamic_loop
        else None
    )

    # quads of slots
    quads = []
    i = 0
    while i < nslot:
        quads.append(list(range(i, min(i + 4, nslot))))
        i += 4

    with TileContext(nc) as tc:
        with (
            tc.tile_pool(name="inp", bufs=2) as inp,
            tc.tile_pool(name="wk", bufs=2) as wk,
            tc.tile_pool(name="psdt", bufs=2, space="PSUM") as psdt,
            tc.tile_pool(name="psmt", bufs=2, space="PSUM") as psmt,
            tc.tile_pool(name="pse", bufs=2, space="PSUM") as pse,
        ):
            import contextlib

            if dynamic_loop:
                nit_t = inp.tile([1, 1], mybir.dt.int32, tag="nit")
                nc.sync.dma_start(nit_t[:], niter_d[:])
                _, (nval,) = nc.values_load_multi_w_load_instructions(
                    nit_t[0:1, 0:1], min_val=1, max_val=8192,
                    skip_runtime_bounds_check=True,
                )
                loop_cm = tc.For_i(0, nval, 1)
            else:
                loop_cm = contextlib.nullcontext()

            with loop_cm:
                cp_t = inp.tile([8, 128 + plan["Wtot"]], BF16, tag="candpix")
                uu_t = inp.tile([128, nslot * 128], F32, tag="uu")
                col_tt = inp.tile([128, nslot * 126], BF16, tag="colors")
                w2_t = inp.tile([128, nslot], F32, tag="w2")
                ident_t = inp.tile([128, 128], F32, tag="ident")
                # candpix alone on SP so the first matmul unblocks fast;
                # small ident/w2 next (gpsimd SWDGE queue unless disabled);
                # colors (needed last, by pC) on the scalar queue
                _dmaq = nc.gpsimd if os.environ.get("DR_SWDGE", "1") == "1" \
                    else nc.scalar
                nc.sync.dma_start(cp_t[:], candpix_d[:])
                _dmaq.dma_start(ident_t[:], ident_d[:])
                _dmaq.dma_start(w2_t[:], w2_d[:])
                half = (nslot // 2) * 128
                nc.sync.dma_start(uu_t[:, :half], uu_d[:, :half])
                _dmaq.dma_start(uu_t[:, half:], uu_d[:, half:])
                halfc = (nslot // 2) * 126
                nc.scalar.dma_start(col_tt[:, :halfc], colors_d[:, :halfc])
                nc.scalar.dma_start(col_tt[:, halfc:], colors_d[:, halfc:])

                def emit_pair(p):
                    """distance matmuls + class reduces + transposes for
                    pair p. Returns mb tile."""
                    W = plan["pair_W"][p]
                    caps = plan["pair_caps"][p]
                    offs = plan["pair_offs"][p]
                    col0 = plan["pair_col0"][p]
                    dt = psdt.tile([128, 1024], F32, tag="dt")
                    for h in range(2):
                        nc.tensor.matmul(
                            dt[:, 512 * h : 512 * h + W],
                            cp_t[0:7, 0:128],
                            cp_t[0:7, col0 + h * W : col0 + (h + 1) * W],
                        )
                    mb = wk.tile([128, 256], F32, tag="mb")
                    mb_v = mb[:].rearrange("p (a c) -> p a c", a=2)
                    dt_v = dt[:].rearrange("p (a b) -> p a b", a=2)
                    rb = 0
                    with tc.tile_wait_until(0.0018 * p):
                        for K in KLIST:
                            cap = caps[K]
                            if cap == 0:
                                continue
                            src = dt_v[
                                :, :, offs[K] : offs[K] + cap * K
                            ].rearrange("p a (n k) -> p a n k", k=K)
                            nc.vector.tensor_reduce(
                                mb_v[:, :, rb : rb + cap], src,
                                axis=mybir.AxisListType.X, op=ALU.max,
                            )
                            rb += cap
                    return mb

                def emit_front(p):
                    """pair p: mm/reduce/transpose + ACT chain through pE."""
                    mb = emit_pair(p)
                    mT = psmt.tile([128, 256], F32, tag="mT")
                    for h in range(2):
                        nc.tensor.transpose(
                            mT[:, 128 * h : 128 * (h + 1)],
                            mb[:, 128 * h : 128 * (h + 1)],
                            ident_t[:],
                        )
                    lnt = wk.tile([128, 256], F32, tag="lnt")
                    s2t = wk.tile([128, 256], F32, tag="s2t")
                    ept = wk.tile([128, 256], F32, tag="ept")
                    spt = wk.tile([128, 256], F32, tag="spt")
                    # one PSUM bank shared by pE (cols 0:256) and pC (256:512)
                    pec = pse.tile([128, 512], F32, tag="pec")
                    pE = pec[:, 0:256]

                    nc.scalar.activation(lnt[:], mT[:], AF.Ln, scale=-4.0)
                    nc.scalar.activation(s2t[:], lnt[:], AF.Exp, scale=0.5)
                    for h in range(2):
                        i = 2 * p + h
                        nc.scalar.activation(
                            ept[:, 128 * h : 128 * (h + 1)],
                            s2t[:, 128 * h : 128 * (h + 1)],
                            AF.Exp, scale=-1.0, bias=w2_t[:, i : i + 1],
                        )
                    nc.scalar.activation(spt[:], ept[:], AF.Ln, bias=1.0)
                    for h in range(2):
                        i = 2 * p + h
                        nc.tensor.matmul(
                            pE[:, 128 * h : 128 * (h + 1)],
                            uu_t[:, 128 * i : 128 * (i + 1)],
                            spt[:, 128 * h : 128 * (h + 1)],
                        )
                    return p, s2t, pec

                def emit_back(p, s2t, pec):
                    t2 = wk.tile([128, 256], F32, tag="t2")
                    wA = wk.tile([128, 256], BF16, tag="wA")
                    outS = wk.tile([128, 256], F32, tag="outS")
                    pC = pec[:, 256:512]
                    nc.vector.tensor_tensor(t2[:], pec[:, 0:256], s2t[:],
                                            ALU.subtract)
                    nc.scalar.activation(wA[:], t2[:], AF.Exp)
                    for h in range(2):
                        i = 2 * p + h
                        nc.tensor.matmul(
                            pC[0:126, 128 * h : 128 * (h + 1)],
                            col_tt[:, 126 * i : 126 * (i + 1)],
                            wA[:, 128 * h : 128 * (h + 1)],
                        )
                    if p % 2 == 0:
                        nc.scalar.activation(outS[0:126, :], pC[0:126, :],
                                             AF.Identity, bias=1.0)
                    else:
                        nc.vector.tensor_scalar(outS[0:126, :], pC[0:126, :],
                                                1.0, None, ALU.add)
                    nc.sync.dma_start(
                        out[0:126, 256 * p : 256 * (p + 1)],
                        outS[0:126, :],
                    )

                # smallest pair first: its reduces gate the first ACT op;
                # largest pairs run in the ACT-saturated middle
                order = sorted(range(npair), key=lambda p: plan["pair_W"][p])
                pending = None
                for p in order:
                    cur = emit_front(p)
                    if pending is not None:
                        emit_back(*pending)
                    pending = cur
                emit_back(*pending)

    _split_excess_waits(nc)
    return nc


def _scatter(plan, core_outs):
    canvas = np.ones((3, CS, CS), np.float32)
    for c in range(NCORES):
        for i in range(plan["nslot"]):
            tiles = plan["slot_tiles"][c][i]
            if not tiles:
                continue
            blk = core_outs[c][:, 128 * i : 128 * (i + 1)]
            for tp, T in enumerate(tiles):
                tyi, txi = divmod(T, NTX)
                canvas[
                    :, tyi * TH : (tyi + 1) * TH, txi * TW : (txi + 1) * TW
                ] = blk[3 * tp : 3 * tp + 3].reshape(3, TH, TW)
    return canvas[None]


def _run(inputs):
    strokes = np.asarray(inputs["strokes"], np.float32)
    widths = np.asarray(inputs["stroke_widths"], np.float32)
    colors = np.asarray(inputs["stroke_colors"], np.float32)
    assert int(inputs["canvas_size"]) == CS

    in_maps, plan = _plan_and_pack(strokes, widths, colors)
    nc = _build_program(plan)
    res = run_bass_kernel_spmd(nc, in_maps, list(range(NCORES)))
    outs = [res.results[c]["out"] for c in range(NCORES)]
    return _scatter(plan, outs), plan, nc, in_maps


def kernel(**inputs):
    out, _, _, _ = _run(inputs)
    return out


def _make_exec(nc, in_maps):
    import jax
    import jax.numpy as jnp
    from jax.sharding import Mesh, PartitionSpec, NamedSharding
    from jax.experimental.shard_map import shard_map
    from concourse import bass2jax

    bass2jax.install_neuronx_cc_hook()
    partition_name = (
        nc.partition_id_tensor.name if nc.partition_id_tensor else None
    )
    in_names, out_names, out_avals = [], [], []
    for alloc in nc.m.functions[0].allocations:
        if not isinstance(alloc, mybir.MemoryLocationSet):
            continue
        name = alloc.memorylocations[0].name
        if alloc.kind == "ExternalInput":
            if name != partition_name:
                in_names.append(name)
        elif alloc.kind == "ExternalOutput":
            out_names.append(name)
            out_avals.append(
                jax.core.ShapedArray(
                    tuple(alloc.tensor_shape), mybir.dt.np(alloc.dtype)
                )
            )
    n_params = len(in_names)
    all_names = in_names + out_names
    if partition_name is not None:
        all_names = all_names + [partition_name]

    def _body(*args):
        operands = list(args)
        if partition_name is not None:
            operands.append(bass2jax.partition_id_tensor())
        outs = bass2jax._bass_exec_p.bind(
            *operands,
            out_avals=tuple(out_avals),
            in_names=tuple(all_names),
            out_names=tuple(out_names),
            lowering_input_output_aliases=(),
            sim_require_finite=True,
            sim_require_nnan=True,
            nc=nc,
        )
        return tuple(outs)

    devices = jax.devices()[:NCORES]
    mesh = Mesh(np.asarray(devices), ("core",))
    n_outs = len(out_names)
    sharded = jax.jit(
        shard_map(
            _body,
            mesh=mesh,
            in_specs=(PartitionSpec("core"),) * (n_params + n_outs),
            out_specs=(PartitionSpec("core"),) * n_outs,
            check_rep=False,
        ),
        donate_argnums=tuple(range(n_params, n_params + n_outs)),
        keep_unused=True,
    )
    concat_in = [
        jnp.asarray(
            np.concatenate([np.asarray(in_maps[c][n]) for c in range(NCORES)], 0)
        )
        for n in in_names
    ]
    zero_shardings = tuple(
        NamedSharding(mesh, PartitionSpec("core")) for _ in out_avals
    )
    zeros_fn = jax.jit(
        lambda: tuple(
            jnp.zeros((a.shape[0] * NCORES,) + a.shape[1:], a.dtype)
            for a in out_avals
        ),
        out_shardings=zero_shardings,
    )

    def run_once():
        return sharded(*concat_in, *zeros_fn())

    return run_once


def timed_run(inputs, reps=10, loop_r=65):
    import jax

    strokes = np.asarray(inputs["strokes"], np.float32)
    widths = np.asarray(inputs["stroke_widths"], np.float32)
    colors = np.asarray(inputs["stroke_colors"], np.float32)
    in_maps, plan = _plan_and_pack(strokes, widths, colors)

    nc = _build_program(plan, dynamic_loop=True)

    def _with_niter(n):
        return [
            {**m, "niter": np.array([[n]], np.int32)} for m in in_maps
        ]

    run1 = _make_exec(nc, _with_niter(1))
    runR = _make_exec(nc, _with_niter(loop_r))

    outs = None
    for _ in range(3):
        outs = run1()
    jax.block_until_ready(outs)
    jax.block_until_ready(runR())

    t1s, tRs = [], []
    for _ in range(reps):
        t0 = time.perf_counter()
        jax.block_until_ready(run1())
        t1s.append(time.perf_counter() - t0)
        t0 = time.perf_counter()
        jax.block_until_ready(runR())
        tRs.append(time.perf_counter() - t0)
    t1 = float(np.median(t1s))
    tR = float(np.median(tRs))
    dt_ns = (tR - t1) / (loop_r - 1) * 1e9
    print(f"  dispatch t1={t1*1e3:.2f}ms tR={tR*1e3:.2f}ms")

    out_global = np.asarray(outs[0])  # [8*128, nslot*128]
    core_outs = [out_global[128 * c : 128 * (c + 1)] for c in range(NCORES)]
    canvas = _scatter(plan, core_outs)
    return canvas, dt_ns, plan


if __name__ == "__main__":
    import reference as ref

    inputs = ref.setup_inputs()
    np_inputs = {
        "strokes": np.asarray(inputs["strokes"]),
        "stroke_widths": np.asarray(inputs["stroke_widths"]),
        "stroke_colors": np.asarray(inputs["stroke_colors"]),
        "canvas_size": inputs["canvas_size"],
    }
    t0 = time.time()
    out, plan, nc, in_maps = _run(np_inputs)
    print("kernel wall time:", time.time() - t0)
    expected = np.asarray(ref.reference(**inputs))
    err = np.abs(out - expected)
    scale = np.abs(expected).max()
    print(f"nslot={plan['nslot']} pair_W={plan['pair_W']}")
    print(f"max abs err: {err.max():.3e}")
    print(f"Relative error: {err.max()/scale:.6e}")
